# revision 28
# baseline (speedup 1.0000x reference)
"""Trainium2 Bass kernel for LoopCoderAttention (sparse_attention).

Head-sharded tensor parallelism over 8 NeuronCores:
  core c owns query heads {2c, 2c+1} and KV head c//2.
All on-device tensors live in transposed [feature, token] layout so every
matmul contracts along the partition dim with zero on-device transposes
(except v, which needs one PE transpose per 128-tile).

v2 layout: the whole matmul datapath is bf16 (fp32 accumulate in PSUM),
which halves input DMA traffic and keeps the PE at 1 cycle/row with fast
weight loads. The local sliding-window pass runs on 128-query subtiles
(2 key-tiles each) instead of 512-query x 5 key-tiles. Global causal
diagonal tiles stream only the unmasked query extent. w_o is prefetched
into SBUF during attention; o_proj computes the high-token half first so
the second all-to-all hides under it.

o_proj: a 2MB AllToAll reshards attention output from head-sharded to
token-sharded; each core then runs the full 2048-deep contraction for its
256-token slice (the "all-reduce" happens inside the matmul accumulation).
"""
import sys
sys.path.insert(0, '/opt/trn_rl_repo')
import numpy as np
import ml_dtypes
import concourse.bass as bass
import concourse.mybir as mybir
import concourse.tile as tile
from concourse import bacc
from concourse.bass_utils import run_bass_kernel_spmd

T = 2048
HID = 2048
HQ = 16
HK = 4
D = 128
WIN = 64
THETA = 10000.0
SCALE = D ** -0.5
NCORES = 8
TCH = 512                 # t-chunk (matmul free dim)
NCH = T // TCH            # 4 chunks
KT = HID // 128           # 16 k-tiles for 2048-deep contractions
ST = T // 128             # 16 s-tiles
TSL = T // NCORES         # 256-token output slice per core
MASKV = -1e9

F32 = mybir.dt.float32
BF16 = mybir.dt.bfloat16
AF = mybir.ActivationFunctionType

_CACHE = {}


def _build():
    nc = bacc.Bacc("TRN2", target_bir_lowering=False, debug=False,
                   num_devices=NCORES)
    HST = nc.dram_tensor("HST", [HID, T], BF16, kind="ExternalInput").ap()
    WQKV = nc.dram_tensor("WQKV", [HID, 512], BF16, kind="ExternalInput").ap()
    KGT = nc.dram_tensor("KGT", [D, T], BF16, kind="ExternalInput").ap()
    VG = nc.dram_tensor("VG", [T, D], BF16, kind="ExternalInput").ap()
    WO = nc.dram_tensor("WO", [HID, HID], BF16, kind="ExternalInput").ap()
    WG = nc.dram_tensor("WG", [D, 2], BF16, kind="ExternalInput").ap()
    BG = nc.dram_tensor("BG", [1, 2], F32, kind="ExternalInput").ap()
    CSF = nc.dram_tensor("CSF", [128, T], BF16, kind="ExternalInput").ap()
    SNF = nc.dram_tensor("SNF", [128, T], BF16, kind="ExternalInput").ap()
    ONES = nc.dram_tensor("ONES", [128, 1], BF16, kind="ExternalInput").ap()
    ONESR = nc.dram_tensor("ONESR", [128, 128], BF16, kind="ExternalInput").ap()
    IDN = nc.dram_tensor("IDN", [128, 128], BF16, kind="ExternalInput").ap()
    MASKD = nc.dram_tensor("MASKD", [128, 128], BF16, kind="ExternalInput").ap()
    MASKA = nc.dram_tensor("MASKA", [128, 128], BF16, kind="ExternalInput").ap()
    MASKB = nc.dram_tensor("MASKB", [128, 128], BF16, kind="ExternalInput").ap()
    OUT = nc.dram_tensor("OUT", [TSL, HID], F32, kind="ExternalOutput").ap()

    with tile.TileContext(nc) as tc:
        # pools are a strict stack: creation order is the reverse of the
        # release order at each phase boundary
        const = tc.alloc_tile_pool(name="const", bufs=1)
        dram = tc.alloc_tile_pool(name="dram", bufs=1, space="DRAM")
        aoutp = tc.alloc_tile_pool(name="aoutp", bufs=3)
        opool = tc.alloc_tile_pool(name="opool", bufs=1)
        osb = tc.alloc_tile_pool(name="osb", bufs=3)
        work = tc.alloc_tile_pool(name="work", bufs=1)
        ropet = tc.alloc_tile_pool(name="ropet", bufs=2)
        combp = tc.alloc_tile_pool(name="combp", bufs=2)
        wqkvp = tc.alloc_tile_pool(name="wqkvp", bufs=1)
        chunkp = tc.alloc_tile_pool(name="chunkp", bufs=2)
        hsp = tc.alloc_tile_pool(name="hsp", bufs=20)
        ps1 = tc.alloc_tile_pool(name="ps1", bufs=7, space="PSUM")

        # ---- phase-1 constants first (critical path to first matmul) ----
        wqkv_sb = wqkvp.tile([128, KT, 512], BF16)
        wqkv_view = WQKV.rearrange("(k p) c -> p k c", p=128)
        hs_tiles = {}
        for k in range(KT):
            nc.sync.dma_start(out=wqkv_sb[:, k, :], in_=wqkv_view[:, k, :])
            hs_t = hsp.tile([128, 2, TCH], BF16, tag="hs_t", name=f"hsp1_{k}")
            # hot half (chunk 3) first so the first k-loop never starves
            nc.sync.dma_start(out=hs_t[:, 1, :],
                              in_=HST[k * 128:(k + 1) * 128, 1536:2048])
            hs_tiles[(1, k)] = hs_t
        for k in range(KT):
            nc.sync.dma_start(out=hs_tiles[(1, k)][:, 0, :],
                              in_=HST[k * 128:(k + 1) * 128, 1024:1536])
        csf_sb = wqkvp.tile([128, T], BF16)
        snf_sb = wqkvp.tile([128, T], BF16)
        idn_sb = wqkvp.tile([128, 128], BF16)
        nc.sync.dma_start(out=csf_sb[:], in_=CSF)
        nc.sync.dma_start(out=snf_sb[:], in_=SNF)
        nc.sync.dma_start(out=idn_sb[:], in_=IDN)
        wg_sb = const.tile([D, 2], BF16)
        nc.sync.dma_start(out=wg_sb[:], in_=WG)
        bg_sb = const.tile([1, 2], F32)
        nc.sync.dma_start(out=bg_sb[:], in_=BG)
        # attention-phase constants (scheduler fills DMA idle time)
        kgt_sb = const.tile([D, T], BF16)
        vg_sb = const.tile([128, ST, D], BF16)
        ones_sb = const.tile([128, 1], BF16)
        onesr_sb = const.tile([128, 128], BF16)
        maskd_sb = const.tile([128, 128], BF16)
        maska_sb = const.tile([128, 128], BF16)
        maskb_sb = const.tile([128, 128], BF16)

        # ---- persistent work tiles (through attention) ----
        qrot = work.tile([128, 2, T], BF16)
        krot = work.tile([128, T], BF16)
        vcur = work.tile([128, ST, D], BF16)   # current v in [s, d] tiles
        # gates staged at the partitions where the softmax-sum rows land:
        # row 0 = g_h0, 32 = g_h1, 64 = 1-g_h0, 96 = 1-g_h1 (per chunk n)
        gstack = work.tile([128, NCH, TCH], F32)
        SMR = (0, 32, 64, 96)

        a2ai_hi = dram.tile([NCORES, 2 * D, TSL // 2], BF16)
        a2ao_hi = dram.tile([NCORES, 2 * D, TSL // 2], BF16)
        a2ai_lo = dram.tile([NCORES, 2 * D, TSL // 2], BF16)
        a2ao_lo = dram.tile([NCORES, 2 * D, TSL // 2], BF16)

        def rope_chunk(dst_full, src, n):
            """dst_full[:, n*TCH:...] = neox-rope of chunk tile src [128, TCH].

            rot = src * [cos;cos] + rot90(src) * [-sin;sin], where rot90 swaps
            the two 64-partition halves (built with two SBUF->SBUF DMAs since
            DVE ops require matching base partitions).
            """
            sl = bass.ds(n * TCH, TCH)
            sr = ropet.tile([128, TCH], BF16, tag="ropesr", name=f"sr{n}")
            nc.sync.dma_start(out=sr[0:64, :], in_=src[64:128, :])
            nc.sync.dma_start(out=sr[64:128, :], in_=src[0:64, :])
            ta = ropet.tile([128, TCH], BF16, tag="ropetmp", name=f"ra{n}")
            tb = ropet.tile([128, TCH], BF16, tag="ropetmp", name=f"rb{n}")
            nc.vector.tensor_mul(ta[:], src[:], csf_sb[:, sl])
            nc.vector.tensor_mul(tb[:], sr[:], snf_sb[:, sl])
            nc.vector.tensor_add(dst_full[:, sl], ta[:], tb[:])

        # ================= phase 1: qkvT = wqkv^T @ hsT =================
        # chunks descend: high-token pair (3,2) first so attention on chunk 3
        # can begin while the low pair computes
        pending_small = []
        for n in reversed(range(NCH)):
            pr, c = n // 2, n % 2
            pss = [ps1.tile([128, TCH], F32, tag="ps1t", name=f"ps1_{n}_{m}")
                   for m in range(4)]
            for k in range(KT):
                if (pr, k) not in hs_tiles:
                    hs_t = hsp.tile([128, 2, TCH], BF16, tag="hs_t",
                                    name=f"hsp{pr}_{k}")
                    nc.sync.dma_start(
                        out=hs_t[:, 1, :],
                        in_=HST[k * 128:(k + 1) * 128, 512:1024])
                    nc.sync.dma_start(
                        out=hs_t[:, 0, :],
                        in_=HST[k * 128:(k + 1) * 128, 0:512])
                    hs_tiles[(pr, k)] = hs_t
                hs_t = hs_tiles[(pr, k)]
                for m in range(4):
                    nc.tensor.matmul(pss[m][:],
                                     wqkv_sb[:, k, m * 128:(m + 1) * 128],
                                     hs_t[:, c, :],
                                     start=(k == 0), stop=(k == KT - 1))
            if pending_small:
                pending_small.pop(0)()
            sl = bass.ds(n * TCH, TCH)
            q0c = chunkp.tile([128, TCH], BF16, tag="q0c")
            q1c = chunkp.tile([128, TCH], BF16, tag="q1c")
            kc = chunkp.tile([128, TCH], BF16, tag="kc")
            vc = chunkp.tile([128, TCH], BF16, tag="vc")
            nc.scalar.activation(q0c[:], pss[0][:], AF.Copy)
            nc.scalar.activation(q1c[:], pss[1][:], AF.Copy)
            nc.scalar.activation(kc[:], pss[2][:], AF.Copy)
            nc.vector.tensor_copy(vc[:], pss[3][:])

            rope_chunk(qrot[:, 0, :], q0c, n)
            rope_chunk(qrot[:, 1, :], q1c, n)
            rope_chunk(krot, kc, n)

            def small_ops(n=n, vc=vc, sl=sl):
                # v transposes + gates for chunk n: emitted one chunk later so
                # the PE stream never waits on the DVE rope/copy latency
                for j in range(4):
                    s = 4 * n + j
                    pt = ps1.tile([128, 128], BF16, tag="ps1g",
                                  name=f"pt{s}", bufs=1)
                    nc.tensor.transpose(pt[:], vc[:, j * 128:(j + 1) * 128],
                                        idn_sb[:])
                    nc.vector.tensor_copy(vcur[:, s, :], pt[:])
                for h in range(2):
                    r = 2 * n + h
                    gp = ps1.tile([1, TCH], F32, tag="ps1g",
                                  name=f"gp{r}", bufs=1)
                    nc.tensor.matmul(gp[:], wg_sb[:, h:h + 1], qrot[:, h, sl],
                                     start=True, stop=True)
                    gst = chunkp.tile([1, TCH], F32, tag="gst", name=f"gst{r}")
                    g1t = chunkp.tile([1, TCH], F32, tag="g1t", name=f"g1t{r}")
                    nc.scalar.activation(gst[:], gp[:], AF.Sigmoid,
                                         bias=bg_sb[0:1, h:h + 1])
                    nc.vector.tensor_scalar(g1t[:], gst[:], -1.0, 1.0,
                                            mybir.AluOpType.mult,
                                            mybir.AluOpType.add)
                    nc.sync.dma_start(out=gstack[SMR[h]:SMR[h] + 1, n, :],
                                      in_=gst[:])
                    nc.sync.dma_start(
                        out=gstack[SMR[2 + h]:SMR[2 + h] + 1, n, :],
                        in_=g1t[:])

            pending_small.append(small_ops)

        for f in pending_small:
            f()
        pending_small.clear()

        nc.sync.dma_start(out=kgt_sb[:], in_=KGT)
        nc.sync.dma_start(out=vg_sb[:],
                          in_=VG.rearrange("(s p) d -> p s d", p=128))
        nc.sync.dma_start(out=ones_sb[:], in_=ONES)
        nc.sync.dma_start(out=onesr_sb[:], in_=ONESR)
        nc.sync.dma_start(out=maskd_sb[:], in_=MASKD)
        nc.sync.dma_start(out=maska_sb[:], in_=MASKA)
        nc.sync.dma_start(out=maskb_sb[:], in_=MASKB)

        ps1.release()
        hsp.release()
        chunkp.release()
        wqkvp.release()

        # w_o prefetch: emitted now so the 8MB streams in during attention,
        # well before the first a2a staging DMA joins the sync queue
        wop = tc.alloc_tile_pool(name="wop", bufs=16)
        wo_tiles = []
        for k in range(KT):
            wo_t = wop.tile([128, HID], BF16, tag="wo", name=f"wo{k}")
            nc.sync.dma_start(out=wo_t[:], in_=WO[k * 128:(k + 1) * 128, :])
            wo_tiles.append(wo_t)

        afull_hi = opool.tile([128, KT, TSL // 2], BF16)
        afull_lo = opool.tile([128, KT, TSL // 2], BF16)

        expp = tc.alloc_tile_pool(name="expp", bufs=6)
        psA = tc.alloc_tile_pool(name="psA", bufs=3, space="PSUM")
        pspv = tc.alloc_tile_pool(name="pspv", bufs=1, space="PSUM")
        pvlp = tc.alloc_tile_pool(name="pvlp", bufs=1, space="PSUM")
        pssm = tc.alloc_tile_pool(name="pssm", bufs=1, space="PSUM")

        # ============ phase 2: attention (global + local) ============
        # chunks descend so the high-token half finishes first and its
        # all-to-all overlaps the low-token half's compute.
        # Per-chunk combine is split: the global half (g/sum_g scaling) hides
        # under the local pass; the local half is deferred into the next
        # chunk's global pass. Per-token scale vectors are broadcast across
        # partitions with a PE outer product (ones[1,128]^T @ agl[1,512])
        # instead of gpsimd, keeping gpsimd free for the all-to-all triggers.
        pend_combine = [None]

        for n in reversed(range(NCH)):
            sl = bass.ds(n * TCH, TCH)
            pv_g = pspv.tile([128, 2, TCH], F32, tag="pv", name=f"pvg{n}")
            pv_l = pvlp.tile([128, 2, TCH], F32, tag="pvl", name=f"pvl{n}")
            # softmax sums: 4 rows (g0,g1,l0,l1) share one PSUM bank at
            # 32-aligned partitions (matmul out base-partition constraint)
            sm = pssm.tile([128, TCH], F32, tag="sm", name=f"sm{n}")
            smsb = combp.tile([128, TCH], F32, tag="smsb", name=f"smsb{n}")
            rcpt = combp.tile([128, TCH], F32, tag="rcpt", name=f"rcpt{n}")
            aglt = combp.tile([128, TCH], BF16, tag="aglt", name=f"aglt{n}")

            # ---- global pass over cached KV (both heads share k/v tiles);
            # diagonal tiles stream only the causally-live query extent
            ns = 4 * n + 4
            for s in range(ns):
                if s == 1 and pend_combine[0] is not None:
                    pend_combine[0]()
                    pend_combine[0] = None
                jo = max(0, (s - 4 * n) * 128)
                mv = bass.ds(n * TCH + jo, TCH - jo)
                qks = []
                for h in range(2):
                    qk = psA.tile([128, TCH], F32, tag="qk",
                                  name=f"qkg{h}_{n}_{s}")
                    nc.tensor.matmul(qk[:, jo:],
                                     kgt_sb[:, s * 128:(s + 1) * 128],
                                     qrot[:, h, mv], start=True, stop=True)
                    qks.append(qk)
                for h in range(2):
                    ex = expp.tile([128, TCH], BF16, tag="ex",
                                   name=f"exg{h}_{n}_{s}")
                    nc.scalar.activation(ex[:, jo:], qks[h][:, jo:],
                                         AF.Exp, scale=SCALE)
                    if s >= 4 * n:
                        # multiplicative 0/1 causal mask on the in-block
                        # triangle (bf16 mul is cheaper than a pre-exp add
                        # and lets exp start straight off the matmul)
                        nc.vector.tensor_mul(ex[:, jo:jo + 128],
                                             ex[:, jo:jo + 128],
                                             maskd_sb[:])
                    qks[h] = ex
                for h in range(2):
                    nc.tensor.matmul(pv_g[:, h, jo:], vg_sb[:, s, :],
                                     qks[h][:, jo:],
                                     start=(s == 0), stop=(s == ns - 1))
                for h in range(2):
                    nc.tensor.matmul(sm[SMR[h]:SMR[h] + 1, jo:], ones_sb[:],
                                     qks[h][:, jo:],
                                     start=(s == 0), stop=(s == ns - 1),
                                     tile_position=(0, SMR[h]))

            if n == 0:
                # gather all-to-all #1 results now: the collective is done (or
                # nearly so), so this never head-blocks the sync DMA queue.
                # Per-k-tile pieces so o_proj can start on the first tile.
                for kk in range(KT):
                    nc.sync.dma_start(
                        out=afull_hi[:, kk, :],
                        in_=a2ao_hi[kk // 2,
                                    (kk % 2) * 128:(kk % 2 + 1) * 128, :])

            # ---- global-combine DVE/ACT half: drain g-sum rows, reciprocal,
            # scale by gate — overlaps the local pass below
            nc.vector.tensor_copy(smsb[0:64, :], sm[0:64, :])
            nc.vector.reciprocal_approx_fast(rcpt[0:64, :], smsb[0:64, :])
            nc.vector.tensor_mul(aglt[0:64, :], rcpt[0:64, :],
                                 gstack[0:64, n, :])

            # ---- local sliding-window pass over current KV: per 128-query
            # subtile only 2 key-tiles are live (band is 64 wide)
            for j in range(4):
                tb_ = 4 * n + j
                ta_ = tb_ - 1
                q0 = n * TCH + 128 * j
                qsl = bass.ds(q0, 128)
                exls = []
                for h in range(2):
                    qkl = psA.tile([128, 2, 128], F32, tag="qk",
                                   name=f"qkl{h}_{n}_{j}")
                    if ta_ >= 0:
                        nc.tensor.matmul(qkl[:, 0, :],
                                         krot[:, ta_ * 128:(ta_ + 1) * 128],
                                         qrot[:, h, qsl],
                                         start=True, stop=True)
                    nc.tensor.matmul(qkl[:, 1, :],
                                     krot[:, tb_ * 128:(tb_ + 1) * 128],
                                     qrot[:, h, qsl],
                                     start=True, stop=True)
                    exls.append(qkl)
                for h in range(2):
                    qkl = exls[h]
                    exl = expp.tile([128, 2, 128], BF16, tag="exl",
                                    name=f"exl{h}_{n}_{j}")
                    t0 = 0 if ta_ >= 0 else 1
                    nc.scalar.activation(exl[:, t0:, :], qkl[:, t0:, :],
                                         AF.Exp, scale=SCALE)
                    if ta_ >= 0:
                        nc.vector.tensor_mul(exl[:, 0, :], exl[:, 0, :],
                                             maska_sb[:])
                    nc.vector.tensor_mul(exl[:, 1, :], exl[:, 1, :],
                                         maskb_sb[:])
                    exls[h] = exl
                osl = bass.ds(128 * j, 128)
                for h in range(2):
                    if ta_ >= 0:
                        nc.tensor.matmul(pv_l[:, h, osl], vcur[:, ta_, :],
                                         exls[h][:, 0, :],
                                         start=True, stop=False)
                    nc.tensor.matmul(pv_l[:, h, osl], vcur[:, tb_, :],
                                     exls[h][:, 1, :],
                                     start=(ta_ < 0), stop=True)
                for h in range(2):
                    p0 = SMR[2 + h]
                    if ta_ >= 0:
                        nc.tensor.matmul(sm[p0:p0 + 1, osl], ones_sb[:],
                                         exls[h][:, 0, :],
                                         start=True, stop=False,
                                         tile_position=(0, p0))
                    nc.tensor.matmul(sm[p0:p0 + 1, osl], ones_sb[:],
                                     exls[h][:, 1, :],
                                     start=(ta_ < 0), stop=True,
                                     tile_position=(0, p0))

            # ---- global-combine tail: broadcast per-token scales and apply
            # to pv_g (frees the pv_g bank for the next chunk). Emitted after
            # the local loop so these long DVE ops never sit ahead of the
            # PV-critical mask multiplies in the DVE FIFO.
            t1s = []
            for h in range(2):
                bcg = psA.tile([128, TCH], F32, tag="qk", name=f"bcg{h}_{n}")
                r0 = SMR[h]
                nc.tensor.matmul(bcg[:], onesr_sb[r0:r0 + 1, :],
                                 aglt[r0:r0 + 1, :],
                                 start=True, stop=True,
                                 tile_position=(r0, 0))
                bcgs = combp.tile([128, TCH], BF16, tag="bcs",
                                  name=f"bcgs{h}_{n}", bufs=4)
                nc.vector.tensor_copy(bcgs[:], bcg[:])
                t1 = combp.tile([128, TCH], F32, tag="comb",
                                name=f"t1_{h}_{n}", bufs=4)
                nc.vector.tensor_mul(t1[:], pv_g[:, h, :], bcgs[:])
                t1s.append(t1)

            # ---- local-combine: deferred into the next chunk's global pass
            # so the scale/broadcast chain hides behind fresh PE work
            def local_combine(n=n, sm=sm, smsb=smsb, rcpt=rcpt, aglt=aglt,
                              pv_l=pv_l, t1s=t1s):
                nc.vector.tensor_copy(smsb[64:128, :], sm[64:128, :])
                # full-partition ops: custom-DVE reciprocal silently no-ops at
                # base partition 64; rows 0-63 recompute harmlessly (their
                # readers, the bcg broadcasts, are already done)
                nc.vector.reciprocal_approx_fast(rcpt[:], smsb[:])
                nc.vector.tensor_mul(aglt[:], rcpt[:], gstack[:, n, :])
                for h in range(2):
                    r = 2 * n + h
                    bcl = psA.tile([128, TCH], F32, tag="qk",
                                   name=f"bcl{h}_{n}")
                    r0 = SMR[2 + h]
                    nc.tensor.matmul(bcl[:], onesr_sb[r0:r0 + 1, :],
                                     aglt[r0:r0 + 1, :],
                                     start=True, stop=True,
                                     tile_position=(r0, 0))
                    bcls = combp.tile([128, TCH], BF16, tag="bcs",
                                      name=f"bcls{h}_{n}", bufs=4)
                    nc.vector.tensor_copy(bcls[:], bcl[:])
                    t2 = combp.tile([128, TCH], F32, tag="comb",
                                    name=f"t2_{h}_{n}", bufs=4)
                    ao = aoutp.tile([128, TCH], BF16, tag="aout",
                                    name=f"ao{r}")
                    nc.vector.tensor_mul(t2[:], pv_l[:, h, :], bcls[:])
                    nc.vector.tensor_add(ao[:], t1s[h][:], t2[:])
                    # ship finished 128-col blocks to a2a staging
                    # token 1024+128c (hi) / 128c (lo) lives in chunk n at
                    # column offset 128jj
                    buf = a2ai_hi if n >= 2 else a2ai_lo
                    c0 = (n - 2) * 4 if n >= 2 else n * 4
                    for jj in range(4):
                        nc.sync.dma_start(
                            out=buf[c0 + jj, h * D:(h + 1) * D, :],
                            in_=ao[:, jj * 128:(jj + 1) * 128])
                if n == 2:
                    # all-to-all #1: high-token halves (overlaps chunks 1,0)
                    nc.gpsimd.collective_compute(
                        "AllToAll", mybir.AluOpType.bypass,
                        replica_groups=[list(range(NCORES))],
                        ins=[a2ai_hi[:].opt()], outs=[a2ao_hi[:].opt()])

            pend_combine[0] = local_combine

        pend_combine[0]()
        pend_combine[0] = None

        pssm.release()
        pvlp.release()
        pspv.release()
        psA.release()
        expp.release()

        # ========= phase 3: all-to-all #2 (low-token halves) =========
        nc.gpsimd.collective_compute(
            "AllToAll", mybir.AluOpType.bypass,
            replica_groups=[list(range(NCORES))],
            ins=[a2ai_lo[:].opt()], outs=[a2ao_lo[:].opt()])
        for kk in range(KT):
            nc.sync.dma_start(
                out=afull_lo[:, kk, :],
                in_=a2ao_lo[kk // 2, (kk % 2) * 128:(kk % 2 + 1) * 128, :])

        pso = tc.alloc_tile_pool(name="pso", bufs=8, space="PSUM")

        # ============ phase 4: o_proj for our token slice ============
        # OUT rows 0-127 = low half-slice, rows 128-255 = high half-slice.
        # hi half first: it only needs all-to-all #1, so the PE works while
        # all-to-all #2 is still in flight
        for tt, afull in ((1, afull_hi), (0, afull_lo)):
            pss2 = [pso.tile([128, TCH], F32, tag="po", name=f"po_{tt}_{e}")
                    for e in range(NCH)]
            for k in range(KT):
                for e in range(NCH):
                    nc.tensor.matmul(pss2[e][:],
                                     afull[:, k, :],
                                     wo_tiles[k][:, e * TCH:(e + 1) * TCH],
                                     start=(k == 0), stop=(k == KT - 1))
            for e in range(NCH):
                ot = osb.tile([128, TCH], F32, tag="ot", name=f"ot{tt}_{e}")
                nc.vector.tensor_copy(ot[:], pss2[e][:])
                nc.sync.dma_start(
                    out=OUT[tt * 128:(tt + 1) * 128,
                            e * TCH:(e + 1) * TCH],
                    in_=ot[:])
        pso.release()
        wop.release()
        combp.release()
        ropet.release()
        work.release()
        osb.release()
        opool.release()
        aoutp.release()
        dram.release()
        const.release()

    nc.compile()
    return nc


def _host_prep(hidden_states, positions, k_global, v_global, w_qkv, w_o,
               w_gate, b_gate):
    """Layout-only host transforms + constant tables -> per-core in_maps."""
    f32 = np.float32
    bf16 = ml_dtypes.bfloat16
    hs = np.asarray(hidden_states, f32)
    pos = np.asarray(positions)
    kg = np.asarray(k_global, f32)
    vg = np.asarray(v_global, f32)
    wqkv = np.asarray(w_qkv, f32)
    wo = np.ascontiguousarray(np.asarray(w_o, f32).astype(bf16))
    wg = np.asarray(w_gate, f32)
    bg = np.asarray(b_gate, f32)

    hst = np.ascontiguousarray(hs.T.astype(bf16))

    half = D // 2
    inv_freq = (THETA ** (-np.arange(half, dtype=f32) / half)).astype(f32)
    ang = pos.astype(f32)[:, None] * inv_freq[None, :]
    cos_t = np.cos(ang).astype(f32).T       # [64, T]
    sin_t = np.sin(ang).astype(f32).T
    csf = np.ascontiguousarray(np.concatenate([cos_t, cos_t], axis=0)).astype(bf16)
    snf = np.ascontiguousarray(np.concatenate([-sin_t, sin_t], axis=0)).astype(bf16)

    p = np.arange(128, dtype=np.int64)[:, None]   # key row within tile
    q = np.arange(128, dtype=np.int64)[None, :]   # query col within block
    # within-block causal triangle for global diagonal tiles (0/1, applied
    # multiplicatively to the exp'd scores)
    maskd = np.where(q >= p, 1.0, 0.0).astype(bf16)
    # local band, key tile one below the query block: delta = q + 128 - p
    maska = np.where(p - q >= 128 - WIN, 1.0, 0.0).astype(bf16)
    # local band, key tile aligned with the query block: delta = q - p
    maskb = np.where((q - p >= 0) & (q - p <= WIN), 1.0, 0.0).astype(bf16)

    ones = np.ones((128, 1), bf16)
    onesr = np.ones((128, 128), bf16)
    idn = np.eye(128, dtype=bf16)

    in_maps = []
    for c in range(NCORES):
        g = c // 2
        wq = wqkv[:, 2 * c * D:(2 * c + 2) * D]
        wk = wqkv[:, HQ * D + g * D:HQ * D + (g + 1) * D]
        wv = wqkv[:, (HQ + HK) * D + g * D:(HQ + HK) * D + (g + 1) * D]
        in_maps.append({
            "HST": hst,
            "WQKV": np.ascontiguousarray(
                np.concatenate([wq, wk, wv], axis=1).astype(bf16)),
            "KGT": np.ascontiguousarray(kg[:, g * D:(g + 1) * D].T.astype(bf16)),
            "VG": np.ascontiguousarray(vg[:, g * D:(g + 1) * D].astype(bf16)),
            "WO": wo,
            "WG": np.ascontiguousarray(wg[:, 2 * c:2 * c + 2].astype(bf16)),
            "BG": np.ascontiguousarray(bg[2 * c:2 * c + 2].reshape(1, 2)),
            "CSF": csf,
            "SNF": snf,
            "ONES": ones,
            "ONESR": onesr,
            "IDN": idn,
            "MASKD": maskd,
            "MASKA": maska,
            "MASKB": maskb,
        })
    return in_maps


def kernel(**inputs):
    if "nc" not in _CACHE:
        _CACHE["nc"] = _build()
    nc = _CACHE["nc"]
    in_maps = _host_prep(**inputs)
    res = run_bass_kernel_spmd(nc, in_maps, core_ids=list(range(NCORES)))
    out = np.empty((T, HID), np.float32)
    for c in range(NCORES):
        o = res.results[c]["OUT"]
        out[128 * c:128 * (c + 1)] = o[0:128]
        out[1024 + 128 * c:1024 + 128 * (c + 1)] = o[128:256]
    return out


# revision 29
# speedup vs baseline: 1.0109x; 1.0109x over previous
"""Trainium2 Bass kernel for LoopCoderAttention (sparse_attention).

Head-sharded tensor parallelism over 8 NeuronCores:
  core c owns query heads {2c, 2c+1} and KV head c//2.
All on-device tensors live in transposed [feature, token] layout so every
matmul contracts along the partition dim with zero on-device transposes
(except v, which needs one PE transpose per 128-tile).

v2 layout: the whole matmul datapath is bf16 (fp32 accumulate in PSUM),
which halves input DMA traffic and keeps the PE at 1 cycle/row with fast
weight loads. The local sliding-window pass runs on 128-query subtiles
(2 key-tiles each) instead of 512-query x 5 key-tiles. Global causal
diagonal tiles stream only the unmasked query extent. w_o is prefetched
into SBUF during attention; o_proj computes the high-token half first so
the second all-to-all hides under it.

o_proj: a 2MB AllToAll reshards attention output from head-sharded to
token-sharded; each core then runs the full 2048-deep contraction for its
256-token slice (the "all-reduce" happens inside the matmul accumulation).
"""
import sys
sys.path.insert(0, '/opt/trn_rl_repo')
import numpy as np
import ml_dtypes
import concourse.bass as bass
import concourse.mybir as mybir
import concourse.tile as tile
from concourse import bacc
from concourse.bass_utils import run_bass_kernel_spmd

T = 2048
HID = 2048
HQ = 16
HK = 4
D = 128
WIN = 64
THETA = 10000.0
SCALE = D ** -0.5
NCORES = 8
TCH = 512                 # t-chunk (matmul free dim)
NCH = T // TCH            # 4 chunks
KT = HID // 128           # 16 k-tiles for 2048-deep contractions
ST = T // 128             # 16 s-tiles
TSL = T // NCORES         # 256-token output slice per core
MASKV = -1e9

F32 = mybir.dt.float32
BF16 = mybir.dt.bfloat16
AF = mybir.ActivationFunctionType

_CACHE = {}


def _build():
    nc = bacc.Bacc("TRN2", target_bir_lowering=False, debug=False,
                   num_devices=NCORES)
    HST = nc.dram_tensor("HST", [HID, T], BF16, kind="ExternalInput").ap()
    WQKV = nc.dram_tensor("WQKV", [HID, 512], BF16, kind="ExternalInput").ap()
    KGT = nc.dram_tensor("KGT", [D, T], BF16, kind="ExternalInput").ap()
    VG = nc.dram_tensor("VG", [T, D], BF16, kind="ExternalInput").ap()
    WO = nc.dram_tensor("WO", [HID, HID], BF16, kind="ExternalInput").ap()
    WG = nc.dram_tensor("WG", [D, 2], BF16, kind="ExternalInput").ap()
    BG = nc.dram_tensor("BG", [1, 2], F32, kind="ExternalInput").ap()
    CSF = nc.dram_tensor("CSF", [128, T], BF16, kind="ExternalInput").ap()
    SNF = nc.dram_tensor("SNF", [128, T], BF16, kind="ExternalInput").ap()
    ONES = nc.dram_tensor("ONES", [128, 1], BF16, kind="ExternalInput").ap()
    ONESR = nc.dram_tensor("ONESR", [128, 128], BF16, kind="ExternalInput").ap()
    IDN = nc.dram_tensor("IDN", [128, 128], BF16, kind="ExternalInput").ap()
    MASKD = nc.dram_tensor("MASKD", [128, 128], BF16, kind="ExternalInput").ap()
    MASKA = nc.dram_tensor("MASKA", [128, 128], BF16, kind="ExternalInput").ap()
    MASKB = nc.dram_tensor("MASKB", [128, 128], BF16, kind="ExternalInput").ap()
    OUT = nc.dram_tensor("OUT", [TSL, HID], F32, kind="ExternalOutput").ap()

    with tile.TileContext(nc) as tc:
        # pools are a strict stack: creation order is the reverse of the
        # release order at each phase boundary
        const = tc.alloc_tile_pool(name="const", bufs=1)
        dram = tc.alloc_tile_pool(name="dram", bufs=1, space="DRAM")
        aoutp = tc.alloc_tile_pool(name="aoutp", bufs=3)
        opool = tc.alloc_tile_pool(name="opool", bufs=1)
        osb = tc.alloc_tile_pool(name="osb", bufs=3)
        work = tc.alloc_tile_pool(name="work", bufs=1)
        ropet = tc.alloc_tile_pool(name="ropet", bufs=2)
        combp = tc.alloc_tile_pool(name="combp", bufs=2)
        wqkvp = tc.alloc_tile_pool(name="wqkvp", bufs=1)
        chunkp = tc.alloc_tile_pool(name="chunkp", bufs=2)
        hsp = tc.alloc_tile_pool(name="hsp", bufs=20)
        ps1 = tc.alloc_tile_pool(name="ps1", bufs=7, space="PSUM")

        # ---- phase-1 constants first (critical path to first matmul) ----
        wqkv_sb = wqkvp.tile([128, KT, 512], BF16)
        wqkv_view = WQKV.rearrange("(k p) c -> p k c", p=128)
        hs_tiles = {}
        for k in range(KT):
            nc.sync.dma_start(out=wqkv_sb[:, k, :], in_=wqkv_view[:, k, :])
            hs_t = hsp.tile([128, 2, TCH], BF16, tag="hs_t", name=f"hsp1_{k}")
            # hot half (chunk 3) first so the first k-loop never starves
            nc.sync.dma_start(out=hs_t[:, 1, :],
                              in_=HST[k * 128:(k + 1) * 128, 1536:2048])
            hs_tiles[(1, k)] = hs_t
        for k in range(KT):
            nc.sync.dma_start(out=hs_tiles[(1, k)][:, 0, :],
                              in_=HST[k * 128:(k + 1) * 128, 1024:1536])
        csf_sb = wqkvp.tile([128, T], BF16)
        snf_sb = wqkvp.tile([128, T], BF16)
        idn_sb = wqkvp.tile([128, 128], BF16)
        nc.sync.dma_start(out=csf_sb[:], in_=CSF)
        nc.sync.dma_start(out=snf_sb[:], in_=SNF)
        nc.sync.dma_start(out=idn_sb[:], in_=IDN)
        wg_sb = const.tile([D, 2], BF16)
        nc.sync.dma_start(out=wg_sb[:], in_=WG)
        bg_sb = const.tile([1, 2], F32)
        nc.sync.dma_start(out=bg_sb[:], in_=BG)
        # attention-phase constants (scheduler fills DMA idle time)
        kgt_sb = const.tile([D, T], BF16)
        vg_sb = const.tile([128, ST, D], BF16)
        ones_sb = const.tile([128, 1], BF16)
        onesr_sb = const.tile([128, 128], BF16)
        maskd_sb = const.tile([128, 128], BF16)
        maska_sb = const.tile([128, 128], BF16)
        maskb_sb = const.tile([128, 128], BF16)

        # ---- persistent work tiles (through attention) ----
        qrot = work.tile([128, 2, T], BF16)
        krot = work.tile([128, T], BF16)
        vcur = work.tile([128, ST, D], BF16)   # current v in [s, d] tiles
        # gates staged at the partitions where the softmax-sum rows land:
        # row 0 = g_h0, 32 = g_h1, 64 = 1-g_h0, 96 = 1-g_h1 (per chunk n)
        gstack = work.tile([128, NCH, TCH], F32)
        SMR = (0, 32, 64, 96)

        a2ai_hi = dram.tile([NCORES, 2 * D, TSL // 2], BF16)
        a2ao_hi = dram.tile([NCORES, 2 * D, TSL // 2], BF16)
        a2ai_lo = dram.tile([NCORES, 2 * D, TSL // 2], BF16)
        a2ao_lo = dram.tile([NCORES, 2 * D, TSL // 2], BF16)

        def rope_chunk(dst_full, src, n):
            """dst_full[:, n*TCH:...] = neox-rope of chunk tile src [128, TCH].

            rot = src * [cos;cos] + rot90(src) * [-sin;sin], where rot90 swaps
            the two 64-partition halves (built with two SBUF->SBUF DMAs since
            DVE ops require matching base partitions).
            """
            sl = bass.ds(n * TCH, TCH)
            sr = ropet.tile([128, TCH], BF16, tag="ropesr", name=f"sr{n}")
            nc.sync.dma_start(out=sr[0:64, :], in_=src[64:128, :])
            nc.sync.dma_start(out=sr[64:128, :], in_=src[0:64, :])
            ta = ropet.tile([128, TCH], BF16, tag="ropetmp", name=f"ra{n}")
            tb = ropet.tile([128, TCH], BF16, tag="ropetmp", name=f"rb{n}")
            nc.vector.tensor_mul(ta[:], src[:], csf_sb[:, sl])
            nc.vector.tensor_mul(tb[:], sr[:], snf_sb[:, sl])
            nc.vector.tensor_add(dst_full[:, sl], ta[:], tb[:])

        # ================= phase 1: qkvT = wqkv^T @ hsT =================
        # chunks descend: high-token pair (3,2) first so attention on chunk 3
        # can begin while the low pair computes
        pending_small = []
        for n in reversed(range(NCH)):
            pr, c = n // 2, n % 2
            pss = [ps1.tile([128, TCH], F32, tag="ps1t", name=f"ps1_{n}_{m}")
                   for m in range(4)]
            for k in range(KT):
                if (pr, k) not in hs_tiles:
                    hs_t = hsp.tile([128, 2, TCH], BF16, tag="hs_t",
                                    name=f"hsp{pr}_{k}")
                    nc.sync.dma_start(
                        out=hs_t[:, 1, :],
                        in_=HST[k * 128:(k + 1) * 128, 512:1024])
                    nc.sync.dma_start(
                        out=hs_t[:, 0, :],
                        in_=HST[k * 128:(k + 1) * 128, 0:512])
                    hs_tiles[(pr, k)] = hs_t
                hs_t = hs_tiles[(pr, k)]
                for m in range(4):
                    nc.tensor.matmul(pss[m][:],
                                     wqkv_sb[:, k, m * 128:(m + 1) * 128],
                                     hs_t[:, c, :],
                                     start=(k == 0), stop=(k == KT - 1))
            if pending_small:
                pending_small.pop(0)()
            sl = bass.ds(n * TCH, TCH)
            q0c = chunkp.tile([128, TCH], BF16, tag="q0c")
            q1c = chunkp.tile([128, TCH], BF16, tag="q1c")
            kc = chunkp.tile([128, TCH], BF16, tag="kc")
            vc = chunkp.tile([128, TCH], BF16, tag="vc")
            nc.scalar.activation(q0c[:], pss[0][:], AF.Copy)
            nc.scalar.activation(q1c[:], pss[1][:], AF.Copy)
            nc.scalar.activation(kc[:], pss[2][:], AF.Copy)
            nc.vector.tensor_copy(vc[:], pss[3][:])

            rope_chunk(qrot[:, 0, :], q0c, n)
            rope_chunk(qrot[:, 1, :], q1c, n)
            rope_chunk(krot, kc, n)

            def small_ops(n=n, vc=vc, sl=sl):
                # v transposes + gates for chunk n: emitted one chunk later so
                # the PE stream never waits on the DVE rope/copy latency
                for j in range(4):
                    s = 4 * n + j
                    pt = ps1.tile([128, 128], BF16, tag="ps1g",
                                  name=f"pt{s}", bufs=1)
                    nc.tensor.transpose(pt[:], vc[:, j * 128:(j + 1) * 128],
                                        idn_sb[:])
                    nc.vector.tensor_copy(vcur[:, s, :], pt[:])
                for h in range(2):
                    r = 2 * n + h
                    gp = ps1.tile([1, TCH], F32, tag="ps1g",
                                  name=f"gp{r}", bufs=1)
                    nc.tensor.matmul(gp[:], wg_sb[:, h:h + 1], qrot[:, h, sl],
                                     start=True, stop=True)
                    gst = chunkp.tile([1, TCH], F32, tag="gst", name=f"gst{r}")
                    g1t = chunkp.tile([1, TCH], F32, tag="g1t", name=f"g1t{r}")
                    nc.scalar.activation(gst[:], gp[:], AF.Sigmoid,
                                         bias=bg_sb[0:1, h:h + 1])
                    nc.vector.tensor_scalar(g1t[:], gst[:], -1.0, 1.0,
                                            mybir.AluOpType.mult,
                                            mybir.AluOpType.add)
                    nc.sync.dma_start(out=gstack[SMR[h]:SMR[h] + 1, n, :],
                                      in_=gst[:])
                    nc.sync.dma_start(
                        out=gstack[SMR[2 + h]:SMR[2 + h] + 1, n, :],
                        in_=g1t[:])

            pending_small.append(small_ops)

        for f in pending_small:
            f()
        pending_small.clear()

        nc.sync.dma_start(out=kgt_sb[:], in_=KGT)
        nc.sync.dma_start(out=vg_sb[:],
                          in_=VG.rearrange("(s p) d -> p s d", p=128))
        nc.sync.dma_start(out=ones_sb[:], in_=ONES)
        nc.sync.dma_start(out=onesr_sb[:], in_=ONESR)
        nc.sync.dma_start(out=maskd_sb[:], in_=MASKD)
        nc.sync.dma_start(out=maska_sb[:], in_=MASKA)
        nc.sync.dma_start(out=maskb_sb[:], in_=MASKB)

        ps1.release()
        hsp.release()
        chunkp.release()
        wqkvp.release()

        # w_o prefetch: emitted now so the 8MB streams in during attention,
        # well before the first a2a staging DMA joins the sync queue
        wop = tc.alloc_tile_pool(name="wop", bufs=16)
        wo_tiles = []
        for k in range(KT):
            wo_t = wop.tile([128, HID], BF16, tag="wo", name=f"wo{k}")
            nc.sync.dma_start(out=wo_t[:], in_=WO[k * 128:(k + 1) * 128, :])
            wo_tiles.append(wo_t)

        afull_hi = opool.tile([128, KT, TSL // 2], BF16)
        afull_lo = opool.tile([128, KT, TSL // 2], BF16)

        expp = tc.alloc_tile_pool(name="expp", bufs=6)
        psA = tc.alloc_tile_pool(name="psA", bufs=3, space="PSUM")
        pspv = tc.alloc_tile_pool(name="pspv", bufs=1, space="PSUM")
        pvlp = tc.alloc_tile_pool(name="pvlp", bufs=1, space="PSUM")
        pssm = tc.alloc_tile_pool(name="pssm", bufs=1, space="PSUM")

        # ============ phase 2: attention (global + local) ============
        # chunks descend so the high-token half finishes first and its
        # all-to-all overlaps the low-token half's compute.
        # Per-chunk combine is split: the global half (g/sum_g scaling) hides
        # under the local pass; the local half is deferred into the next
        # chunk's global pass. Per-token scale vectors are broadcast across
        # partitions with a PE outer product (ones[1,128]^T @ agl[1,512])
        # instead of gpsimd, keeping gpsimd free for the all-to-all triggers.
        pend_combine = [None]

        for n in reversed(range(NCH)):
            sl = bass.ds(n * TCH, TCH)
            pv_g = pspv.tile([128, 2, TCH], F32, tag="pv", name=f"pvg{n}")
            pv_l = pvlp.tile([128, 2, TCH], F32, tag="pvl", name=f"pvl{n}")
            # softmax sums: 4 rows (g0,g1,l0,l1) share one PSUM bank at
            # 32-aligned partitions (matmul out base-partition constraint)
            sm = pssm.tile([128, TCH], F32, tag="sm", name=f"sm{n}")
            smsb = combp.tile([128, TCH], F32, tag="smsb", name=f"smsb{n}")
            rcpt = combp.tile([128, TCH], F32, tag="rcpt", name=f"rcpt{n}")
            aglt = combp.tile([128, TCH], BF16, tag="aglt", name=f"aglt{n}")

            # ---- global pass over cached KV (both heads share k/v tiles);
            # diagonal tiles stream only the causally-live query extent
            ns = 4 * n + 4

            # software pipeline: emit step s's QK+exp, then step s-1's PV/SM
            # (whose exp finished during this step's QKs) so the PE never
            # waits on the ACT engine inside a step
            def emit_pvsm_g(s, jo, exs):
                for h in range(2):
                    nc.tensor.matmul(pv_g[:, h, jo:], vg_sb[:, s, :],
                                     exs[h][:, jo:],
                                     start=(s == 0), stop=(s == ns - 1))
                for h in range(2):
                    nc.tensor.matmul(sm[SMR[h]:SMR[h] + 1, jo:], ones_sb[:],
                                     exs[h][:, jo:],
                                     start=(s == 0), stop=(s == ns - 1),
                                     tile_position=(0, SMR[h]))

            prev_g = None
            for s in range(ns):
                if s == 1 and pend_combine[0] is not None:
                    pend_combine[0]()
                    pend_combine[0] = None
                jo = max(0, (s - 4 * n) * 128)
                mv = bass.ds(n * TCH + jo, TCH - jo)
                qks = []
                for h in range(2):
                    qk = psA.tile([128, TCH], F32, tag="qk",
                                  name=f"qkg{h}_{n}_{s}")
                    nc.tensor.matmul(qk[:, jo:],
                                     kgt_sb[:, s * 128:(s + 1) * 128],
                                     qrot[:, h, mv], start=True, stop=True)
                    qks.append(qk)
                for h in range(2):
                    ex = expp.tile([128, TCH], BF16, tag="ex",
                                   name=f"exg{h}_{n}_{s}")
                    nc.scalar.activation(ex[:, jo:], qks[h][:, jo:],
                                         AF.Exp, scale=SCALE)
                    if s >= 4 * n:
                        # multiplicative 0/1 causal mask on the in-block
                        # triangle (bf16 mul is cheaper than a pre-exp add
                        # and lets exp start straight off the matmul)
                        nc.vector.tensor_mul(ex[:, jo:jo + 128],
                                             ex[:, jo:jo + 128],
                                             maskd_sb[:])
                    qks[h] = ex
                if prev_g is not None:
                    emit_pvsm_g(*prev_g)
                prev_g = (s, jo, qks)
            emit_pvsm_g(*prev_g)

            if n == 0:
                # gather all-to-all #1 results now: the collective is done (or
                # nearly so), so this never head-blocks the sync DMA queue.
                # Per-k-tile pieces so o_proj can start on the first tile.
                for kk in range(KT):
                    nc.sync.dma_start(
                        out=afull_hi[:, kk, :],
                        in_=a2ao_hi[kk // 2,
                                    (kk % 2) * 128:(kk % 2 + 1) * 128, :])

            # ---- global-combine DVE/ACT half: drain g-sum rows, reciprocal,
            # scale by gate — overlaps the local pass below
            nc.vector.tensor_copy(smsb[0:64, :], sm[0:64, :])
            nc.vector.reciprocal_approx_fast(rcpt[0:64, :], smsb[0:64, :])
            nc.vector.tensor_mul(aglt[0:64, :], rcpt[0:64, :],
                                 gstack[0:64, n, :])

            # ---- local sliding-window pass over current KV: per 128-query
            # subtile only 2 key-tiles are live (band is 64 wide)
            def emit_pvsm_l(j, ta_, tb_, exls):
                osl = bass.ds(128 * j, 128)
                for h in range(2):
                    if ta_ >= 0:
                        nc.tensor.matmul(pv_l[:, h, osl], vcur[:, ta_, :],
                                         exls[h][:, 0, :],
                                         start=True, stop=False)
                    nc.tensor.matmul(pv_l[:, h, osl], vcur[:, tb_, :],
                                     exls[h][:, 1, :],
                                     start=(ta_ < 0), stop=True)
                for h in range(2):
                    p0 = SMR[2 + h]
                    if ta_ >= 0:
                        nc.tensor.matmul(sm[p0:p0 + 1, osl], ones_sb[:],
                                         exls[h][:, 0, :],
                                         start=True, stop=False,
                                         tile_position=(0, p0))
                    nc.tensor.matmul(sm[p0:p0 + 1, osl], ones_sb[:],
                                     exls[h][:, 1, :],
                                     start=(ta_ < 0), stop=True,
                                     tile_position=(0, p0))

            prev_l = None
            for j in range(4):
                tb_ = 4 * n + j
                ta_ = tb_ - 1
                q0 = n * TCH + 128 * j
                qsl = bass.ds(q0, 128)
                exls = []
                for h in range(2):
                    qkl = psA.tile([128, 2, 128], F32, tag="qk",
                                   name=f"qkl{h}_{n}_{j}")
                    if ta_ >= 0:
                        nc.tensor.matmul(qkl[:, 0, :],
                                         krot[:, ta_ * 128:(ta_ + 1) * 128],
                                         qrot[:, h, qsl],
                                         start=True, stop=True)
                    nc.tensor.matmul(qkl[:, 1, :],
                                     krot[:, tb_ * 128:(tb_ + 1) * 128],
                                     qrot[:, h, qsl],
                                     start=True, stop=True)
                    exls.append(qkl)
                for h in range(2):
                    qkl = exls[h]
                    exl = expp.tile([128, 2, 128], BF16, tag="exl",
                                    name=f"exl{h}_{n}_{j}")
                    t0 = 0 if ta_ >= 0 else 1
                    nc.scalar.activation(exl[:, t0:, :], qkl[:, t0:, :],
                                         AF.Exp, scale=SCALE)
                    if ta_ >= 0:
                        nc.vector.tensor_mul(exl[:, 0, :], exl[:, 0, :],
                                             maska_sb[:])
                    nc.vector.tensor_mul(exl[:, 1, :], exl[:, 1, :],
                                         maskb_sb[:])
                    exls[h] = exl
                if prev_l is not None:
                    emit_pvsm_l(*prev_l)
                prev_l = (j, ta_, tb_, exls)
            emit_pvsm_l(*prev_l)

            # ---- global-combine tail: broadcast per-token scales and apply
            # to pv_g (frees the pv_g bank for the next chunk). Emitted after
            # the local loop so these long DVE ops never sit ahead of the
            # PV-critical mask multiplies in the DVE FIFO.
            t1s = []
            for h in range(2):
                bcg = psA.tile([128, TCH], F32, tag="qk", name=f"bcg{h}_{n}")
                r0 = SMR[h]
                nc.tensor.matmul(bcg[:], onesr_sb[r0:r0 + 1, :],
                                 aglt[r0:r0 + 1, :],
                                 start=True, stop=True,
                                 tile_position=(r0, 0))
                bcgs = combp.tile([128, TCH], BF16, tag="bcs",
                                  name=f"bcgs{h}_{n}", bufs=4)
                nc.vector.tensor_copy(bcgs[:], bcg[:])
                t1 = combp.tile([128, TCH], F32, tag="comb",
                                name=f"t1_{h}_{n}", bufs=4)
                nc.vector.tensor_mul(t1[:], pv_g[:, h, :], bcgs[:])
                t1s.append(t1)

            # ---- local-combine: deferred into the next chunk's global pass
            # so the scale/broadcast chain hides behind fresh PE work
            def local_combine(n=n, sm=sm, smsb=smsb, rcpt=rcpt, aglt=aglt,
                              pv_l=pv_l, t1s=t1s):
                nc.vector.tensor_copy(smsb[64:128, :], sm[64:128, :])
                # full-partition ops: custom-DVE reciprocal silently no-ops at
                # base partition 64; rows 0-63 recompute harmlessly (their
                # readers, the bcg broadcasts, are already done)
                nc.vector.reciprocal_approx_fast(rcpt[:], smsb[:])
                nc.vector.tensor_mul(aglt[:], rcpt[:], gstack[:, n, :])
                for h in range(2):
                    r = 2 * n + h
                    bcl = psA.tile([128, TCH], F32, tag="qk",
                                   name=f"bcl{h}_{n}")
                    r0 = SMR[2 + h]
                    nc.tensor.matmul(bcl[:], onesr_sb[r0:r0 + 1, :],
                                     aglt[r0:r0 + 1, :],
                                     start=True, stop=True,
                                     tile_position=(r0, 0))
                    bcls = combp.tile([128, TCH], BF16, tag="bcs",
                                      name=f"bcls{h}_{n}", bufs=4)
                    nc.vector.tensor_copy(bcls[:], bcl[:])
                    t2 = combp.tile([128, TCH], F32, tag="comb",
                                    name=f"t2_{h}_{n}", bufs=4)
                    ao = aoutp.tile([128, TCH], BF16, tag="aout",
                                    name=f"ao{r}")
                    nc.vector.tensor_mul(t2[:], pv_l[:, h, :], bcls[:])
                    nc.vector.tensor_add(ao[:], t1s[h][:], t2[:])
                    # ship finished 128-col blocks to a2a staging
                    # token 1024+128c (hi) / 128c (lo) lives in chunk n at
                    # column offset 128jj
                    buf = a2ai_hi if n >= 2 else a2ai_lo
                    c0 = (n - 2) * 4 if n >= 2 else n * 4
                    for jj in range(4):
                        nc.sync.dma_start(
                            out=buf[c0 + jj, h * D:(h + 1) * D, :],
                            in_=ao[:, jj * 128:(jj + 1) * 128])
                if n == 2:
                    # all-to-all #1: high-token halves (overlaps chunks 1,0)
                    nc.gpsimd.collective_compute(
                        "AllToAll", mybir.AluOpType.bypass,
                        replica_groups=[list(range(NCORES))],
                        ins=[a2ai_hi[:].opt()], outs=[a2ao_hi[:].opt()])

            pend_combine[0] = local_combine

        pend_combine[0]()
        pend_combine[0] = None

        pssm.release()
        pvlp.release()
        pspv.release()
        psA.release()
        expp.release()

        # ========= phase 3: all-to-all #2 (low-token halves) =========
        nc.gpsimd.collective_compute(
            "AllToAll", mybir.AluOpType.bypass,
            replica_groups=[list(range(NCORES))],
            ins=[a2ai_lo[:].opt()], outs=[a2ao_lo[:].opt()])
        for kk in range(KT):
            nc.sync.dma_start(
                out=afull_lo[:, kk, :],
                in_=a2ao_lo[kk // 2, (kk % 2) * 128:(kk % 2 + 1) * 128, :])

        pso = tc.alloc_tile_pool(name="pso", bufs=8, space="PSUM")

        # ============ phase 4: o_proj for our token slice ============
        # OUT rows 0-127 = low half-slice, rows 128-255 = high half-slice.
        # hi half first: it only needs all-to-all #1, so the PE works while
        # all-to-all #2 is still in flight
        for tt, afull in ((1, afull_hi), (0, afull_lo)):
            pss2 = [pso.tile([128, TCH], F32, tag="po", name=f"po_{tt}_{e}")
                    for e in range(NCH)]
            for k in range(KT):
                for e in range(NCH):
                    nc.tensor.matmul(pss2[e][:],
                                     afull[:, k, :],
                                     wo_tiles[k][:, e * TCH:(e + 1) * TCH],
                                     start=(k == 0), stop=(k == KT - 1))
            for e in range(NCH):
                ot = osb.tile([128, TCH], F32, tag="ot", name=f"ot{tt}_{e}")
                nc.vector.tensor_copy(ot[:], pss2[e][:])
                nc.sync.dma_start(
                    out=OUT[tt * 128:(tt + 1) * 128,
                            e * TCH:(e + 1) * TCH],
                    in_=ot[:])
        pso.release()
        wop.release()
        combp.release()
        ropet.release()
        work.release()
        osb.release()
        opool.release()
        aoutp.release()
        dram.release()
        const.release()

    nc.compile()
    return nc


def _host_prep(hidden_states, positions, k_global, v_global, w_qkv, w_o,
               w_gate, b_gate):
    """Layout-only host transforms + constant tables -> per-core in_maps."""
    f32 = np.float32
    bf16 = ml_dtypes.bfloat16
    hs = np.asarray(hidden_states, f32)
    pos = np.asarray(positions)
    kg = np.asarray(k_global, f32)
    vg = np.asarray(v_global, f32)
    wqkv = np.asarray(w_qkv, f32)
    wo = np.ascontiguousarray(np.asarray(w_o, f32).astype(bf16))
    wg = np.asarray(w_gate, f32)
    bg = np.asarray(b_gate, f32)

    hst = np.ascontiguousarray(hs.T.astype(bf16))

    half = D // 2
    inv_freq = (THETA ** (-np.arange(half, dtype=f32) / half)).astype(f32)
    ang = pos.astype(f32)[:, None] * inv_freq[None, :]
    cos_t = np.cos(ang).astype(f32).T       # [64, T]
    sin_t = np.sin(ang).astype(f32).T
    csf = np.ascontiguousarray(np.concatenate([cos_t, cos_t], axis=0)).astype(bf16)
    snf = np.ascontiguousarray(np.concatenate([-sin_t, sin_t], axis=0)).astype(bf16)

    p = np.arange(128, dtype=np.int64)[:, None]   # key row within tile
    q = np.arange(128, dtype=np.int64)[None, :]   # query col within block
    # within-block causal triangle for global diagonal tiles (0/1, applied
    # multiplicatively to the exp'd scores)
    maskd = np.where(q >= p, 1.0, 0.0).astype(bf16)
    # local band, key tile one below the query block: delta = q + 128 - p
    maska = np.where(p - q >= 128 - WIN, 1.0, 0.0).astype(bf16)
    # local band, key tile aligned with the query block: delta = q - p
    maskb = np.where((q - p >= 0) & (q - p <= WIN), 1.0, 0.0).astype(bf16)

    ones = np.ones((128, 1), bf16)
    onesr = np.ones((128, 128), bf16)
    idn = np.eye(128, dtype=bf16)

    in_maps = []
    for c in range(NCORES):
        g = c // 2
        wq = wqkv[:, 2 * c * D:(2 * c + 2) * D]
        wk = wqkv[:, HQ * D + g * D:HQ * D + (g + 1) * D]
        wv = wqkv[:, (HQ + HK) * D + g * D:(HQ + HK) * D + (g + 1) * D]
        in_maps.append({
            "HST": hst,
            "WQKV": np.ascontiguousarray(
                np.concatenate([wq, wk, wv], axis=1).astype(bf16)),
            "KGT": np.ascontiguousarray(kg[:, g * D:(g + 1) * D].T.astype(bf16)),
            "VG": np.ascontiguousarray(vg[:, g * D:(g + 1) * D].astype(bf16)),
            "WO": wo,
            "WG": np.ascontiguousarray(wg[:, 2 * c:2 * c + 2].astype(bf16)),
            "BG": np.ascontiguousarray(bg[2 * c:2 * c + 2].reshape(1, 2)),
            "CSF": csf,
            "SNF": snf,
            "ONES": ones,
            "ONESR": onesr,
            "IDN": idn,
            "MASKD": maskd,
            "MASKA": maska,
            "MASKB": maskb,
        })
    return in_maps


def kernel(**inputs):
    if "nc" not in _CACHE:
        _CACHE["nc"] = _build()
    nc = _CACHE["nc"]
    in_maps = _host_prep(**inputs)
    res = run_bass_kernel_spmd(nc, in_maps, core_ids=list(range(NCORES)))
    out = np.empty((T, HID), np.float32)
    for c in range(NCORES):
        o = res.results[c]["OUT"]
        out[128 * c:128 * (c + 1)] = o[0:128]
        out[1024 + 128 * c:1024 + 128 * (c + 1)] = o[128:256]
    return out


# revision 31
# speedup vs baseline: 1.0505x; 1.0392x over previous
"""Trainium2 Bass kernel for LoopCoderAttention (sparse_attention).

Head-sharded tensor parallelism over 8 NeuronCores:
  core c owns query heads {2c, 2c+1} and KV head c//2.
All on-device tensors live in transposed [feature, token] layout so every
matmul contracts along the partition dim with zero on-device transposes
(except v, which needs one PE transpose per 128-tile).

v2 layout: the whole matmul datapath is bf16 (fp32 accumulate in PSUM),
which halves input DMA traffic and keeps the PE at 1 cycle/row with fast
weight loads. The local sliding-window pass runs on 128-query subtiles
(2 key-tiles each) instead of 512-query x 5 key-tiles. Global causal
diagonal tiles stream only the unmasked query extent. w_o is prefetched
into SBUF during attention; o_proj computes the high-token half first so
the second all-to-all hides under it.

o_proj: a 2MB AllToAll reshards attention output from head-sharded to
token-sharded; each core then runs the full 2048-deep contraction for its
256-token slice (the "all-reduce" happens inside the matmul accumulation).
"""
import sys
sys.path.insert(0, '/opt/trn_rl_repo')
import numpy as np
import ml_dtypes
import concourse.bass as bass
import concourse.mybir as mybir
import concourse.tile as tile
from concourse import bacc
from concourse.bass_utils import run_bass_kernel_spmd

T = 2048
HID = 2048
HQ = 16
HK = 4
D = 128
WIN = 64
THETA = 10000.0
SCALE = D ** -0.5
NCORES = 8
TCH = 512                 # t-chunk (matmul free dim)
NCH = T // TCH            # 4 chunks
KT = HID // 128           # 16 k-tiles for 2048-deep contractions
ST = T // 128             # 16 s-tiles
TSL = T // NCORES         # 256-token output slice per core
MASKV = -1e9

F32 = mybir.dt.float32
BF16 = mybir.dt.bfloat16
AF = mybir.ActivationFunctionType

_CACHE = {}


def _build():
    nc = bacc.Bacc("TRN2", target_bir_lowering=False, debug=False,
                   num_devices=NCORES)
    HST = nc.dram_tensor("HST", [HID, T], BF16, kind="ExternalInput").ap()
    WQKV = nc.dram_tensor("WQKV", [HID, 512], BF16, kind="ExternalInput").ap()
    KGT = nc.dram_tensor("KGT", [D, T], BF16, kind="ExternalInput").ap()
    VG = nc.dram_tensor("VG", [T, D], BF16, kind="ExternalInput").ap()
    WO = nc.dram_tensor("WO", [HID, HID], BF16, kind="ExternalInput").ap()
    WG = nc.dram_tensor("WG", [D, 2], BF16, kind="ExternalInput").ap()
    BG = nc.dram_tensor("BG", [1, 2], F32, kind="ExternalInput").ap()
    CSF = nc.dram_tensor("CSF", [128, T], BF16, kind="ExternalInput").ap()
    SNF = nc.dram_tensor("SNF", [128, T], BF16, kind="ExternalInput").ap()
    ONES = nc.dram_tensor("ONES", [128, 1], BF16, kind="ExternalInput").ap()
    ONESR = nc.dram_tensor("ONESR", [128, 128], BF16, kind="ExternalInput").ap()
    IDN = nc.dram_tensor("IDN", [128, 128], BF16, kind="ExternalInput").ap()
    MASKD = nc.dram_tensor("MASKD", [128, 128], BF16, kind="ExternalInput").ap()
    MASKL = nc.dram_tensor("MASKL", [128, 256], BF16, kind="ExternalInput").ap()
    OUT = nc.dram_tensor("OUT", [TSL, HID], F32, kind="ExternalOutput").ap()

    with tile.TileContext(nc) as tc:
        # pools are a strict stack: creation order is the reverse of the
        # release order at each phase boundary
        const = tc.alloc_tile_pool(name="const", bufs=1)
        dram = tc.alloc_tile_pool(name="dram", bufs=1, space="DRAM")
        aoutp = tc.alloc_tile_pool(name="aoutp", bufs=3)
        opool = tc.alloc_tile_pool(name="opool", bufs=1)
        osb = tc.alloc_tile_pool(name="osb", bufs=3)
        work = tc.alloc_tile_pool(name="work", bufs=1)
        ropet = tc.alloc_tile_pool(name="ropet", bufs=2)
        combp = tc.alloc_tile_pool(name="combp", bufs=2)
        wqkvp = tc.alloc_tile_pool(name="wqkvp", bufs=1)
        chunkp = tc.alloc_tile_pool(name="chunkp", bufs=2)
        hsp = tc.alloc_tile_pool(name="hsp", bufs=20)
        ps1 = tc.alloc_tile_pool(name="ps1", bufs=7, space="PSUM")

        # ---- phase-1 constants first (critical path to first matmul) ----
        wqkv_sb = wqkvp.tile([128, KT, 512], BF16)
        wqkv_view = WQKV.rearrange("(k p) c -> p k c", p=128)
        hs_tiles = {}
        for k in range(KT):
            nc.sync.dma_start(out=wqkv_sb[:, k, :], in_=wqkv_view[:, k, :])
            hs_t = hsp.tile([128, 2, TCH], BF16, tag="hs_t", name=f"hsp1_{k}")
            # hot half (chunk 3) first so the first k-loop never starves
            nc.sync.dma_start(out=hs_t[:, 1, :],
                              in_=HST[k * 128:(k + 1) * 128, 1536:2048])
            hs_tiles[(1, k)] = hs_t
        for k in range(KT):
            nc.sync.dma_start(out=hs_tiles[(1, k)][:, 0, :],
                              in_=HST[k * 128:(k + 1) * 128, 1024:1536])
        csf_sb = wqkvp.tile([128, T], BF16)
        snf_sb = wqkvp.tile([128, T], BF16)
        idn_sb = wqkvp.tile([128, 128], BF16)
        nc.sync.dma_start(out=csf_sb[:], in_=CSF)
        nc.sync.dma_start(out=snf_sb[:], in_=SNF)
        nc.sync.dma_start(out=idn_sb[:], in_=IDN)
        wg_sb = const.tile([D, 2], BF16)
        nc.sync.dma_start(out=wg_sb[:], in_=WG)
        bg_sb = const.tile([1, 2], F32)
        nc.sync.dma_start(out=bg_sb[:], in_=BG)
        # attention-phase constants (scheduler fills DMA idle time)
        kgt_sb = const.tile([D, T], BF16)
        vg_sb = const.tile([128, ST, D], BF16)
        ones_sb = const.tile([128, 1], BF16)
        onesr_sb = const.tile([128, 128], BF16)
        maskd_sb = const.tile([128, 128], BF16)
        maskl_sb = const.tile([128, 256], BF16)

        # ---- persistent work tiles (through attention) ----
        qrot = work.tile([128, 2, T], BF16)
        krot = work.tile([128, T], BF16)
        vcur = work.tile([128, ST, D], BF16)   # current v in [s, d] tiles
        # gates staged at the partitions where the softmax-sum rows land:
        # row 0 = g_h0, 32 = g_h1, 64 = 1-g_h0, 96 = 1-g_h1 (per chunk n)
        gstack = work.tile([128, NCH, TCH], F32)
        SMR = (0, 32, 64, 96)

        a2ai_hi = dram.tile([NCORES, 2 * D, TSL // 2], BF16)
        a2ao_hi = dram.tile([NCORES, 2 * D, TSL // 2], BF16)
        a2ai_lo = dram.tile([NCORES, 2 * D, TSL // 2], BF16)
        a2ao_lo = dram.tile([NCORES, 2 * D, TSL // 2], BF16)

        def rope_chunk(dst_full, src, n):
            """dst_full[:, n*TCH:...] = neox-rope of chunk tile src [128, TCH].

            rot = src * [cos;cos] + rot90(src) * [-sin;sin], where rot90 swaps
            the two 64-partition halves (built with two SBUF->SBUF DMAs since
            DVE ops require matching base partitions).
            """
            sl = bass.ds(n * TCH, TCH)
            sr = ropet.tile([128, TCH], BF16, tag="ropesr", name=f"sr{n}")
            nc.sync.dma_start(out=sr[0:64, :], in_=src[64:128, :])
            nc.sync.dma_start(out=sr[64:128, :], in_=src[0:64, :])
            ta = ropet.tile([128, TCH], BF16, tag="ropetmp", name=f"ra{n}")
            tb = ropet.tile([128, TCH], BF16, tag="ropetmp", name=f"rb{n}")
            nc.vector.tensor_mul(ta[:], src[:], csf_sb[:, sl])
            nc.vector.tensor_mul(tb[:], sr[:], snf_sb[:, sl])
            nc.vector.tensor_add(dst_full[:, sl], ta[:], tb[:])

        # ================= phase 1: qkvT = wqkv^T @ hsT =================
        # chunks descend: high-token pair (3,2) first so attention on chunk 3
        # can begin while the low pair computes
        pending_small = []
        for n in reversed(range(NCH)):
            pr, c = n // 2, n % 2
            pss = [ps1.tile([128, TCH], F32, tag="ps1t", name=f"ps1_{n}_{m}")
                   for m in range(4)]
            for k in range(KT):
                if (pr, k) not in hs_tiles:
                    hs_t = hsp.tile([128, 2, TCH], BF16, tag="hs_t",
                                    name=f"hsp{pr}_{k}")
                    nc.sync.dma_start(
                        out=hs_t[:, 1, :],
                        in_=HST[k * 128:(k + 1) * 128, 512:1024])
                    nc.sync.dma_start(
                        out=hs_t[:, 0, :],
                        in_=HST[k * 128:(k + 1) * 128, 0:512])
                    hs_tiles[(pr, k)] = hs_t
                hs_t = hs_tiles[(pr, k)]
                for m in range(4):
                    nc.tensor.matmul(pss[m][:],
                                     wqkv_sb[:, k, m * 128:(m + 1) * 128],
                                     hs_t[:, c, :],
                                     start=(k == 0), stop=(k == KT - 1))
            if pending_small:
                pending_small.pop(0)()
            sl = bass.ds(n * TCH, TCH)
            q0c = chunkp.tile([128, TCH], BF16, tag="q0c")
            q1c = chunkp.tile([128, TCH], BF16, tag="q1c")
            kc = chunkp.tile([128, TCH], BF16, tag="kc")
            vc = chunkp.tile([128, TCH], BF16, tag="vc")
            nc.scalar.activation(q0c[:], pss[0][:], AF.Copy)
            nc.scalar.activation(q1c[:], pss[1][:], AF.Copy)
            nc.scalar.activation(kc[:], pss[2][:], AF.Copy)
            nc.vector.tensor_copy(vc[:], pss[3][:])

            rope_chunk(qrot[:, 0, :], q0c, n)
            rope_chunk(qrot[:, 1, :], q1c, n)
            rope_chunk(krot, kc, n)

            def small_ops(n=n, vc=vc, sl=sl):
                # v transposes + gates for chunk n: emitted one chunk later so
                # the PE stream never waits on the DVE rope/copy latency
                for j in range(4):
                    s = 4 * n + j
                    pt = ps1.tile([128, 128], BF16, tag="ps1g",
                                  name=f"pt{s}", bufs=1)
                    nc.tensor.transpose(pt[:], vc[:, j * 128:(j + 1) * 128],
                                        idn_sb[:])
                    nc.vector.tensor_copy(vcur[:, s, :], pt[:])
                for h in range(2):
                    r = 2 * n + h
                    gp = ps1.tile([1, TCH], F32, tag="ps1g",
                                  name=f"gp{r}", bufs=1)
                    nc.tensor.matmul(gp[:], wg_sb[:, h:h + 1], qrot[:, h, sl],
                                     start=True, stop=True)
                    gst = chunkp.tile([1, TCH], F32, tag="gst", name=f"gst{r}")
                    g1t = chunkp.tile([1, TCH], F32, tag="g1t", name=f"g1t{r}")
                    nc.scalar.activation(gst[:], gp[:], AF.Sigmoid,
                                         bias=bg_sb[0:1, h:h + 1])
                    nc.vector.tensor_scalar(g1t[:], gst[:], -1.0, 1.0,
                                            mybir.AluOpType.mult,
                                            mybir.AluOpType.add)
                    nc.sync.dma_start(out=gstack[SMR[h]:SMR[h] + 1, n, :],
                                      in_=gst[:])
                    nc.sync.dma_start(
                        out=gstack[SMR[2 + h]:SMR[2 + h] + 1, n, :],
                        in_=g1t[:])

            pending_small.append(small_ops)

        for f in pending_small:
            f()
        pending_small.clear()

        nc.sync.dma_start(out=kgt_sb[:], in_=KGT)
        nc.sync.dma_start(out=vg_sb[:],
                          in_=VG.rearrange("(s p) d -> p s d", p=128))
        nc.sync.dma_start(out=ones_sb[:], in_=ONES)
        nc.sync.dma_start(out=onesr_sb[:], in_=ONESR)
        nc.sync.dma_start(out=maskd_sb[:], in_=MASKD)
        nc.sync.dma_start(out=maskl_sb[:], in_=MASKL)

        ps1.release()
        hsp.release()
        chunkp.release()
        wqkvp.release()

        # w_o prefetch: emitted now so the 8MB streams in during attention,
        # well before the first a2a staging DMA joins the sync queue
        wop = tc.alloc_tile_pool(name="wop", bufs=16)
        wo_tiles = []
        for k in range(KT):
            wo_t = wop.tile([128, HID], BF16, tag="wo", name=f"wo{k}")
            nc.sync.dma_start(out=wo_t[:], in_=WO[k * 128:(k + 1) * 128, :])
            wo_tiles.append(wo_t)

        afull_hi = opool.tile([128, KT, TSL // 2], BF16)
        afull_lo = opool.tile([128, KT, TSL // 2], BF16)

        expp = tc.alloc_tile_pool(name="expp", bufs=6)
        psA = tc.alloc_tile_pool(name="psA", bufs=3, space="PSUM")
        pspv = tc.alloc_tile_pool(name="pspv", bufs=1, space="PSUM")
        pvlp = tc.alloc_tile_pool(name="pvlp", bufs=1, space="PSUM")
        pssm = tc.alloc_tile_pool(name="pssm", bufs=1, space="PSUM")

        # ============ phase 2: attention (global + local) ============
        # chunks descend so the high-token half finishes first and its
        # all-to-all overlaps the low-token half's compute.
        # Per-chunk combine is split: the global half (g/sum_g scaling) hides
        # under the local pass; the local half is deferred into the next
        # chunk's global pass. Per-token scale vectors are broadcast across
        # partitions with a PE outer product (ones[1,128]^T @ agl[1,512])
        # instead of gpsimd, keeping gpsimd free for the all-to-all triggers.
        pend_combine = [None]

        for n in reversed(range(NCH)):
            sl = bass.ds(n * TCH, TCH)
            pv_g = pspv.tile([128, 2, TCH], F32, tag="pv", name=f"pvg{n}")
            pv_l = pvlp.tile([128, 2, TCH], F32, tag="pvl", name=f"pvl{n}")
            # softmax sums: 4 rows (g0,g1,l0,l1) share one PSUM bank at
            # 32-aligned partitions (matmul out base-partition constraint)
            sm = pssm.tile([128, TCH], F32, tag="sm", name=f"sm{n}")
            smsb = combp.tile([128, TCH], F32, tag="smsb", name=f"smsb{n}")
            rcpt = combp.tile([128, TCH], F32, tag="rcpt", name=f"rcpt{n}")
            aglt = combp.tile([128, TCH], BF16, tag="aglt", name=f"aglt{n}")

            # ---- global pass over cached KV (both heads share k/v tiles);
            # diagonal tiles stream only the causally-live query extent
            ns = 4 * n + 4

            # software pipeline: emit step s's QK+exp, then step s-1's PV/SM
            # (whose exp finished during this step's QKs) so the PE never
            # waits on the ACT engine inside a step
            def emit_pvsm_g(s, jo, exs):
                for h in range(2):
                    nc.tensor.matmul(pv_g[:, h, jo:], vg_sb[:, s, :],
                                     exs[h][:, jo:],
                                     start=(s == 0), stop=(s == ns - 1))
                for h in range(2):
                    nc.tensor.matmul(sm[SMR[h]:SMR[h] + 1, jo:], ones_sb[:],
                                     exs[h][:, jo:],
                                     start=(s == 0), stop=(s == ns - 1),
                                     tile_position=(0, SMR[h]))

            prev_g = None
            for s in range(ns):
                if s == 1 and pend_combine[0] is not None:
                    pend_combine[0]()
                    pend_combine[0] = None
                jo = max(0, (s - 4 * n) * 128)
                mv = bass.ds(n * TCH + jo, TCH - jo)
                qks = []
                for h in range(2):
                    qk = psA.tile([128, TCH], F32, tag="qk",
                                  name=f"qkg{h}_{n}_{s}")
                    nc.tensor.matmul(qk[:, jo:],
                                     kgt_sb[:, s * 128:(s + 1) * 128],
                                     qrot[:, h, mv], start=True, stop=True)
                    qks.append(qk)
                for h in range(2):
                    ex = expp.tile([128, TCH], BF16, tag="ex",
                                   name=f"exg{h}_{n}_{s}")
                    nc.scalar.activation(ex[:, jo:], qks[h][:, jo:],
                                         AF.Exp, scale=SCALE)
                    if s >= 4 * n:
                        # multiplicative 0/1 causal mask on the in-block
                        # triangle (bf16 mul is cheaper than a pre-exp add
                        # and lets exp start straight off the matmul)
                        nc.vector.tensor_mul(ex[:, jo:jo + 128],
                                             ex[:, jo:jo + 128],
                                             maskd_sb[:])
                    qks[h] = ex
                if prev_g is not None:
                    emit_pvsm_g(*prev_g)
                prev_g = (s, jo, qks)
            emit_pvsm_g(*prev_g)

            if n == 0:
                # gather all-to-all #1 results now: the collective is done (or
                # nearly so), so this never head-blocks the sync DMA queue.
                # Per-k-tile pieces so o_proj can start on the first tile.
                for kk in range(KT):
                    nc.sync.dma_start(
                        out=afull_hi[:, kk, :],
                        in_=a2ao_hi[kk // 2,
                                    (kk % 2) * 128:(kk % 2 + 1) * 128, :])

            # ---- global-combine DVE/ACT half: drain g-sum rows, reciprocal,
            # scale by gate — overlaps the local pass below
            nc.vector.tensor_copy(smsb[0:64, :], sm[0:64, :])
            nc.vector.reciprocal_approx_fast(rcpt[0:64, :], smsb[0:64, :])
            nc.vector.tensor_mul(aglt[0:64, :], rcpt[0:64, :],
                                 gstack[0:64, n, :])

            # ---- local sliding-window pass over current KV: per 128-query
            # subtile only 2 key-tiles are live (band is 64 wide)
            def emit_pvsm_l(t, e0, w, st, sp, exls):
                osl = bass.ds(e0, w)
                for h in range(2):
                    nc.tensor.matmul(pv_l[:, h, osl], vcur[:, t, :],
                                     exls[h][:, 0:w],
                                     start=st, stop=sp,
                                     skip_group_check=True)
                for h in range(2):
                    p0 = SMR[2 + h]
                    nc.tensor.matmul(sm[p0:p0 + 1, osl], ones_sb[:],
                                     exls[h][:, 0:w],
                                     start=st, stop=sp,
                                     tile_position=(0, p0),
                                     skip_group_check=True)

            # one matmul per key tile over a padded 256-query extent with a
            # single shift-invariant band mask. Tiles 4n and 4n+2 are emitted
            # first with start=True: their extents exactly partition [0,512)
            # so the remaining tiles accumulate with start=False.
            # start=True only on the first tile: it row-clears the whole
            # bank, so tile 4n+2's disjoint region write-if-cleans correctly
            # and the overlapping odd tiles accumulate
            lt = [(4 * n, 0, 256, 0, True, False),
                  (4 * n + 2, 256, 256, 0, False, False),
                  (4 * n - 1, 0, 64, 128, False, True),
                  (4 * n + 1, 128, 192, 0, False, True),
                  (4 * n + 3, 384, 128, 0, False, True)]
            prev_l = None
            for (t, e0, w, m0, st, sp) in lt:
                if t < 0:
                    continue
                qsl = bass.ds(n * TCH + e0, w)
                exls = []
                for h in range(2):
                    qkl = psA.tile([128, 256], F32, tag="qk",
                                   name=f"qkl{h}_{n}_{t}")
                    nc.tensor.matmul(qkl[:, 0:w],
                                     krot[:, t * 128:(t + 1) * 128],
                                     qrot[:, h, qsl],
                                     start=True, stop=True)
                    exls.append(qkl)
                for h in range(2):
                    exl = expp.tile([128, 256], BF16, tag="exl",
                                    name=f"exl{h}_{n}_{t}")
                    nc.scalar.activation(exl[:, 0:w], exls[h][:, 0:w],
                                         AF.Exp, scale=SCALE)
                    nc.vector.tensor_mul(exl[:, 0:w], exl[:, 0:w],
                                         maskl_sb[:, m0:m0 + w])
                    exls[h] = exl
                if prev_l is not None:
                    emit_pvsm_l(*prev_l)
                prev_l = (t, e0, w, st, sp, exls)
            emit_pvsm_l(*prev_l)

            # ---- global-combine tail: broadcast per-token scales and apply
            # to pv_g (frees the pv_g bank for the next chunk). Emitted after
            # the local loop so these long DVE ops never sit ahead of the
            # PV-critical mask multiplies in the DVE FIFO.
            t1s = []
            for h in range(2):
                bcg = psA.tile([128, TCH], F32, tag="qk", name=f"bcg{h}_{n}")
                r0 = SMR[h]
                nc.tensor.matmul(bcg[:], onesr_sb[r0:r0 + 1, :],
                                 aglt[r0:r0 + 1, :],
                                 start=True, stop=True,
                                 tile_position=(r0, 0))
                bcgs = combp.tile([128, TCH], BF16, tag="bcs",
                                  name=f"bcgs{h}_{n}", bufs=4)
                nc.vector.tensor_copy(bcgs[:], bcg[:])
                t1 = combp.tile([128, TCH], F32, tag="comb",
                                name=f"t1_{h}_{n}", bufs=4)
                nc.vector.tensor_mul(t1[:], pv_g[:, h, :], bcgs[:])
                t1s.append(t1)

            # ---- local-combine: deferred into the next chunk's global pass
            # so the scale/broadcast chain hides behind fresh PE work
            def local_combine(n=n, sm=sm, smsb=smsb, rcpt=rcpt, aglt=aglt,
                              pv_l=pv_l, t1s=t1s):
                nc.vector.tensor_copy(smsb[64:128, :], sm[64:128, :])
                # full-partition ops: custom-DVE reciprocal silently no-ops at
                # base partition 64; rows 0-63 recompute harmlessly (their
                # readers, the bcg broadcasts, are already done)
                nc.vector.reciprocal_approx_fast(rcpt[:], smsb[:])
                nc.vector.tensor_mul(aglt[:], rcpt[:], gstack[:, n, :])
                for h in range(2):
                    r = 2 * n + h
                    bcl = psA.tile([128, TCH], F32, tag="qk",
                                   name=f"bcl{h}_{n}")
                    r0 = SMR[2 + h]
                    nc.tensor.matmul(bcl[:], onesr_sb[r0:r0 + 1, :],
                                     aglt[r0:r0 + 1, :],
                                     start=True, stop=True,
                                     tile_position=(r0, 0))
                    bcls = combp.tile([128, TCH], BF16, tag="bcs",
                                      name=f"bcls{h}_{n}", bufs=4)
                    nc.vector.tensor_copy(bcls[:], bcl[:])
                    t2 = combp.tile([128, TCH], F32, tag="comb",
                                    name=f"t2_{h}_{n}", bufs=4)
                    ao = aoutp.tile([128, TCH], BF16, tag="aout",
                                    name=f"ao{r}")
                    nc.vector.tensor_mul(t2[:], pv_l[:, h, :], bcls[:])
                    nc.vector.tensor_add(ao[:], t1s[h][:], t2[:])
                    # ship finished 128-col blocks to a2a staging
                    # token 1024+128c (hi) / 128c (lo) lives in chunk n at
                    # column offset 128jj
                    buf = a2ai_hi if n >= 2 else a2ai_lo
                    c0 = (n - 2) * 4 if n >= 2 else n * 4
                    for jj in range(4):
                        nc.sync.dma_start(
                            out=buf[c0 + jj, h * D:(h + 1) * D, :],
                            in_=ao[:, jj * 128:(jj + 1) * 128])
                if n == 2:
                    # all-to-all #1: high-token halves (overlaps chunks 1,0)
                    nc.gpsimd.collective_compute(
                        "AllToAll", mybir.AluOpType.bypass,
                        replica_groups=[list(range(NCORES))],
                        ins=[a2ai_hi[:].opt()], outs=[a2ao_hi[:].opt()])

            pend_combine[0] = local_combine

        pend_combine[0]()
        pend_combine[0] = None

        pssm.release()
        pvlp.release()
        pspv.release()
        psA.release()
        expp.release()

        # ========= phase 3: all-to-all #2 (low-token halves) =========
        nc.gpsimd.collective_compute(
            "AllToAll", mybir.AluOpType.bypass,
            replica_groups=[list(range(NCORES))],
            ins=[a2ai_lo[:].opt()], outs=[a2ao_lo[:].opt()])
        for kk in range(KT):
            nc.sync.dma_start(
                out=afull_lo[:, kk, :],
                in_=a2ao_lo[kk // 2, (kk % 2) * 128:(kk % 2 + 1) * 128, :])

        pso = tc.alloc_tile_pool(name="pso", bufs=8, space="PSUM")

        # ============ phase 4: o_proj for our token slice ============
        # OUT rows 0-127 = low half-slice, rows 128-255 = high half-slice.
        # hi half first: it only needs all-to-all #1, so the PE works while
        # all-to-all #2 is still in flight
        for tt, afull in ((1, afull_hi), (0, afull_lo)):
            pss2 = [pso.tile([128, TCH], F32, tag="po", name=f"po_{tt}_{e}")
                    for e in range(NCH)]
            for k in range(KT):
                for e in range(NCH):
                    nc.tensor.matmul(pss2[e][:],
                                     afull[:, k, :],
                                     wo_tiles[k][:, e * TCH:(e + 1) * TCH],
                                     start=(k == 0), stop=(k == KT - 1))
            for e in range(NCH):
                ot = osb.tile([128, TCH], F32, tag="ot", name=f"ot{tt}_{e}")
                nc.vector.tensor_copy(ot[:], pss2[e][:])
                nc.sync.dma_start(
                    out=OUT[tt * 128:(tt + 1) * 128,
                            e * TCH:(e + 1) * TCH],
                    in_=ot[:])
        pso.release()
        wop.release()
        combp.release()
        ropet.release()
        work.release()
        osb.release()
        opool.release()
        aoutp.release()
        dram.release()
        const.release()

    nc.compile()
    return nc


def _host_prep(hidden_states, positions, k_global, v_global, w_qkv, w_o,
               w_gate, b_gate):
    """Layout-only host transforms + constant tables -> per-core in_maps."""
    f32 = np.float32
    bf16 = ml_dtypes.bfloat16
    hs = np.asarray(hidden_states, f32)
    pos = np.asarray(positions)
    kg = np.asarray(k_global, f32)
    vg = np.asarray(v_global, f32)
    wqkv = np.asarray(w_qkv, f32)
    wo = np.ascontiguousarray(np.asarray(w_o, f32).astype(bf16))
    wg = np.asarray(w_gate, f32)
    bg = np.asarray(b_gate, f32)

    hst = np.ascontiguousarray(hs.T.astype(bf16))

    half = D // 2
    inv_freq = (THETA ** (-np.arange(half, dtype=f32) / half)).astype(f32)
    ang = pos.astype(f32)[:, None] * inv_freq[None, :]
    cos_t = np.cos(ang).astype(f32).T       # [64, T]
    sin_t = np.sin(ang).astype(f32).T
    csf = np.ascontiguousarray(np.concatenate([cos_t, cos_t], axis=0)).astype(bf16)
    snf = np.ascontiguousarray(np.concatenate([-sin_t, sin_t], axis=0)).astype(bf16)

    p = np.arange(128, dtype=np.int64)[:, None]   # key row within tile
    q = np.arange(128, dtype=np.int64)[None, :]   # query col within block
    # within-block causal triangle for global diagonal tiles (0/1, applied
    # multiplicatively to the exp'd scores)
    maskd = np.where(q >= p, 1.0, 0.0).astype(bf16)
    # canonical local band mask: key row k vs query offset e within a
    # 256-query extent starting at the key tile's base
    e = np.arange(256, dtype=np.int64)[None, :]
    maskl = np.where((e - p >= 0) & (e - p <= WIN), 1.0, 0.0).astype(bf16)

    ones = np.ones((128, 1), bf16)
    onesr = np.ones((128, 128), bf16)
    idn = np.eye(128, dtype=bf16)

    in_maps = []
    for c in range(NCORES):
        g = c // 2
        wq = wqkv[:, 2 * c * D:(2 * c + 2) * D]
        wk = wqkv[:, HQ * D + g * D:HQ * D + (g + 1) * D]
        wv = wqkv[:, (HQ + HK) * D + g * D:(HQ + HK) * D + (g + 1) * D]
        in_maps.append({
            "HST": hst,
            "WQKV": np.ascontiguousarray(
                np.concatenate([wq, wk, wv], axis=1).astype(bf16)),
            "KGT": np.ascontiguousarray(kg[:, g * D:(g + 1) * D].T.astype(bf16)),
            "VG": np.ascontiguousarray(vg[:, g * D:(g + 1) * D].astype(bf16)),
            "WO": wo,
            "WG": np.ascontiguousarray(wg[:, 2 * c:2 * c + 2].astype(bf16)),
            "BG": np.ascontiguousarray(bg[2 * c:2 * c + 2].reshape(1, 2)),
            "CSF": csf,
            "SNF": snf,
            "ONES": ones,
            "ONESR": onesr,
            "IDN": idn,
            "MASKD": maskd,
            "MASKL": maskl,
        })
    return in_maps


def kernel(**inputs):
    if "nc" not in _CACHE:
        _CACHE["nc"] = _build()
    nc = _CACHE["nc"]
    in_maps = _host_prep(**inputs)
    res = run_bass_kernel_spmd(nc, in_maps, core_ids=list(range(NCORES)))
    out = np.empty((T, HID), np.float32)
    for c in range(NCORES):
        o = res.results[c]["OUT"]
        out[128 * c:128 * (c + 1)] = o[0:128]
        out[1024 + 128 * c:1024 + 128 * (c + 1)] = o[128:256]
    return out
